# revision 1
# baseline (speedup 1.0000x reference)
"""BitLinear (ternary-quantized linear) kernel for Trainium2, 8 NeuronCores.

Reference computation:
    scale = mean(|W|);  Wq = round(W / (scale + 1e-5));  y = (x @ Wq^T) * scale

Distribution (2x4 grid over 8 cores):
  - batch/sequence dim (8192 rows of x) split 2 ways  -> ri = core // 4
  - out_features dim (4096 rows of W) split 4 ways    -> ci = core % 4
  Each core computes y block [4096 s, 1024 o].
  scale needs the *global* mean(|W|): each core reduces a distinct 1/8 slice
  of W (the `wred` input) and a tiny [128] AllReduce combines the partials.

Host side does layout only: passes x^T / W^T slices (the contraction dim i
must be the SBUF partition dim on both matmul operands), and stitches the
output blocks back together (no host transpose of the output needed).

All FLOPs (reduction, quantization, matmul, rescale) run on device.
"""

import os
import sys
import types

import numpy as np


def _ensure_axon_hooks_module():
    """Some images lack ``antenv.axon_hooks``; ``run_bass_kernel_spmd`` imports
    it unconditionally when tracing is requested. Install a no-op fallback so a
    BASS_TRACE=1 environment degrades to "no trace" instead of crashing."""
    try:
        import antenv.axon_hooks  # noqa: F401
        return
    except ImportError:
        pass
    try:
        import antenv
    except ImportError:
        return
    mod = types.ModuleType("antenv.axon_hooks")
    mod._hook = None

    def set_axon_ntff_profile_hook(h):
        mod._hook = h

    def get_axon_ntff_profile_hook():
        return mod._hook

    mod.set_axon_ntff_profile_hook = set_axon_ntff_profile_hook
    mod.get_axon_ntff_profile_hook = get_axon_ntff_profile_hook
    sys.modules["antenv.axon_hooks"] = mod
    antenv.axon_hooks = mod


_ensure_axon_hooks_module()

# ---- problem constants (hardcoded per contract) ----
B, SEQ, I_DIM, O_DIM = 4, 2048, 4096, 4096
S_TOT = B * SEQ            # 8192
R_CORES, C_CORES = 2, 4    # grid: batch x out_features
N_CORES = R_CORES * C_CORES
S_CORE = S_TOT // R_CORES  # 4096 sequence rows per core
O_CORE = O_DIM // C_CORES  # 1024 output features per core
P = 128
KP = I_DIM // P            # 32 contraction chunks
S_BLK = 256                # s columns per x load block
N_SBLK = S_CORE // S_BLK   # 16
W_RED = O_DIM // N_CORES   # 512: columns of W^T reduced per core for mean|W|
WCH = 2                    # ko chunks per W staging tile ([128, 2, 1024] = 1 MB)
N_WT = KP // WCH           # 16 stage/quantize tiles
N_ACT_Q = 6                # how many quantize tiles go to the scalar engine
MAGIC = 1.5 * (2.0 ** 23)  # fp32 round-to-nearest-even trick constant
EPS = 1e-5
# cross-core combine for the |W| partial sums:
#   "cc"     = ncfw AllGather collective (~90-120us first-collective floor)
#   "remote" = direct SDMA peer writes + pinned semaphore (~5us) — blocked:
#              Tile's scheduling simulator deadlocks on waits that only
#              remote cores satisfy, so this needs a raw-bacc kernel
GATHER = os.environ.get("BITLIN_GATHER", "cc")

_nc_cache = {}


def _build_kernel():
    import concourse.mybir as mybir
    import concourse.tile as tile
    from concourse import bacc
    from concourse.tile import add_dep_helper

    f32 = mybir.dt.float32
    bf16 = mybir.dt.bfloat16
    Alu = mybir.AluOpType
    Act = mybir.ActivationFunctionType

    nc = bacc.Bacc(
        "TRN2",
        target_bir_lowering=False,
        debug=False,
        enable_asserts=False,
        num_devices=N_CORES,
    )

    xT = nc.dram_tensor("xT", [I_DIM, S_CORE], f32, kind="ExternalInput")
    wT = nc.dram_tensor("wT", [I_DIM, O_CORE], f32, kind="ExternalInput")
    wred = nc.dram_tensor("wred", [I_DIM, W_RED], f32, kind="ExternalInput")
    y = nc.dram_tensor("y", [S_CORE, O_CORE], f32, kind="ExternalOutput")

    xT_r = xT.ap().rearrange("(ko p) s -> p ko s", p=P)    # [128, 32, 4096]
    wT_r = wT.ap().rearrange("(ko p) o -> p ko o", p=P)    # [128, 32, 1024]
    wred_r = wred.ap().rearrange("(ko p) o -> p ko o", p=P)  # [128, 32, 512]
    y_ap = y.ap()

    with tile.TileContext(nc) as tc:
        with (
            tc.tile_pool(name="const", bufs=1) as const_pool,
            tc.tile_pool(name="stats", bufs=1) as stats,
            tc.tile_pool(name="wstage", bufs=7) as wstage,
            tc.tile_pool(name="wq", bufs=1) as wq_pool,
            tc.tile_pool(name="xbf", bufs=3) as xbf_pool,
            tc.tile_pool(name="yout", bufs=3) as yout_pool,
            tc.tile_pool(name="psum_s", bufs=1, space="PSUM") as psum_s,
            tc.tile_pool(name="psum_mm", bufs=3, space="PSUM") as psum_mm,
            tc.tile_pool(name="dram", bufs=1, space="DRAM") as dram_pool,
        ):
            # ---------- Phase A: per-partition partial sums of |wred| ----------
            # split across DVE (tensor_reduce) and ACT (Abs + accum_out) so the
            # AllReduce trigger fires as early as possible
            n_rtiles = KP // 4  # 8 tiles [128, 4, 512] = 1 MB each
            red_all = stats.tile([P, n_rtiles], f32)
            for t in range(n_rtiles):
                wt = wstage.tile([P, 4, W_RED], f32, tag="wstage")
                nc.sync.dma_start(wt[:], wred_r[:, t * 4 : (t + 1) * 4, :])
                if t % 2 == 0:
                    nc.vector.tensor_reduce(
                        red_all[:, t : t + 1],
                        wt[:],
                        axis=mybir.AxisListType.XY,
                        op=Alu.add,
                        apply_absolute_value=True,
                    )
                else:
                    nc.scalar.activation(
                        wt[:], wt[:], Act.Abs, accum_out=red_all[:, t : t + 1]
                    )
            acc = stats.tile([P, 1], f32)
            nc.vector.tensor_reduce(
                acc[:], red_all[:], axis=mybir.AxisListType.X, op=Alu.add
            )

            # ---------- Phase B: gather the [128] partial sums across cores ----------
            if GATHER == "remote":
                # Direct SDMA peer-SBUF writes, XOR-relative addressing:
                # this core's frame with delta d lands in peer (self^d)'s recv
                # column d; by XOR symmetry receiver r's column d holds the
                # partial of core (r^d) — all 8 slots distinct, all addressing
                # compile-time (SPMD-clean). No ncfw/TOPSP involvement.
                rsem = nc.alloc_semaphore("bl_rsem")
                lsem = nc.alloc_semaphore("bl_lsem")
                clr = nc.gpsimd.sem_clear(rsem)
                recv = stats.tile([P, N_CORES], f32)
                nc.vector.tensor_copy(recv[:, 0:1], acc[:])  # self slot
                prev = clr
                for delta in range(1, N_CORES):
                    rdests = [None] * N_CORES
                    rdests[delta] = (0, delta)
                    pr = nc.gpsimd.remote_dma_broadcast(
                        out_ap=recv[:, delta : delta + 1],
                        in_ap=acc[:],
                        remote_sem=rsem,
                        local_sem=lsem,
                        rdests=rdests,
                    )
                    add_dep_helper(pr.ins, prev.ins, sync=False,
                                   reason="remote gather prep order")
                    prev = pr
                gate = nc.gpsimd.trigger_dma(count=None)
                acc_r = stats.tile([P, 1], f32)
                # critical section: the wait is satisfied by REMOTE cores'
                # sem increments, which the scheduling sim cannot model
                with tc.tile_critical():
                    nc.vector.wait_ge(rsem, 2 * (N_CORES - 1))
                    nc.vector.tensor_reduce(
                        acc_r[:], recv[:], axis=mybir.AxisListType.X, op=Alu.add
                    )
                bounce_dma = gate  # wT staging waits on this below
            else:
                # ncfw AllGather fallback (AG has a lower floor than AR; the
                # cross-rank sum folds into the broadcast matmul below)
                cc_in = dram_pool.tile([P, 1], f32)
                cc_out = dram_pool.tile([N_CORES * P, 1], f32, addr_space="Shared")
                bounce_dma = nc.sync.dma_start(cc_in[:], acc[:])
                gate = nc.gpsimd.collective_compute(
                    "AllGather",
                    Alu.bypass,
                    replica_groups=[list(range(N_CORES))],
                    ins=[cc_in.opt()],
                    outs=[cc_out.opt()],
                )
                # read back as [128, 8]: partition p, free r <- dram[r*128 + p].
                # Keep this per-partition tree reduction: a flat 1024-element
                # sequential sum lands measurably further from the reference's
                # fp32 summation (rel err 1.7e-3 -> 1.0e-2).
                acc_g = stats.tile([P, N_CORES], f32)
                nc.sync.dma_start(
                    acc_g[:], cc_out.rearrange("(r p) one -> p (r one)", p=P)
                )
                acc_r = stats.tile([P, 1], f32)
                nc.vector.tensor_reduce(
                    acc_r[:], acc_g[:], axis=mybir.AxisListType.X, op=Alu.add
                )

            # ---------- Phase C: scale scalars, broadcast to all partitions ----------
            # global sum broadcast: ones^T @ acc_r -> every partition = full sum
            K_B = acc_r.shape[0]  # 1 (cc path) or 128 (remote path)
            ones_b = const_pool.tile([K_B, P], f32)
            nc.vector.memset(ones_b[:], 1.0)
            ps_b = psum_s.tile([P, 1], f32)
            nc.tensor.matmul(ps_b[:], lhsT=ones_b[:], rhs=acc_r[:], start=True, stop=True)

            inv_numel = 1.0 / (float(I_DIM) * float(O_DIM))
            # sinv first: it gates quantization (scale_t is only needed at
            # output eviction, much later)
            seps_t = stats.tile([P, 1], f32)   # scale + eps
            nc.vector.tensor_scalar(
                seps_t[:], ps_b[:], inv_numel, EPS, op0=Alu.mult, op1=Alu.add
            )
            sinv_t = stats.tile([P, 1], f32)   # 1 / (scale + eps)
            nc.vector.reciprocal(sinv_t[:], seps_t[:])
            scale_t = stats.tile([P, 1], f32)  # mean(|W|)
            nc.vector.tensor_scalar_mul(scale_t[:], ps_b[:], inv_numel)

            # ---------- Phase D: quantize W -> bf16 integers (DVE + ACT split) ----------
            # wq split into N_WT tiles so matmuls can start on partially-done W
            wq_tiles = []
            for t in range(N_WT):
                wq_tiles.append(
                    wq_pool.tile([P, WCH, O_CORE], bf16, tag=f"wq{t}", name=f"wq{t}")
                )
            # stage + quantize in PE-consumption order so early-needed tiles
            # (including the ACT-engine ones) are staged/quantized first.
            # ACT handles tiles 10..15; DVE handles 0..9.
            t_order = [0, 10, 1, 2, 11, 3, 12, 4, 5, 13, 6, 14, 7, 8, 15, 9]
            first_done = False
            for pos, t in enumerate(t_order):
                wt = wstage.tile([P, WCH, O_CORE], f32, tag="wstage")
                dma_eng = nc.sync if pos % 2 == 0 else nc.scalar
                dma = dma_eng.dma_start(wt[:], wT_r[:, t * WCH : (t + 1) * WCH, :])
                if not first_done:
                    first_done = True
                    # keep pass-1 (wred) DMAs exclusive on the queue until the
                    # collective input is on its way
                    add_dep_helper(dma.ins, bounce_dma.ins, sync=False,
                                   reason="stage wT after AR input bounce")
                if t < N_WT - N_ACT_Q:
                    # wn = W * (1/(scale+eps)) + MAGIC  (fp32, in place)
                    nc.vector.tensor_scalar(
                        wt[:], wt[:], sinv_t[:], MAGIC, op0=Alu.mult, op1=Alu.add
                    )
                    # wq = (wn - MAGIC) cast to bf16  (exact small integers)
                    nc.vector.tensor_scalar_sub(wq_tiles[t][:], wt[:], MAGIC)
                else:
                    nc.scalar.activation(
                        wt[:], wt[:], Act.Copy, bias=MAGIC, scale=sinv_t[:]
                    )
                    nc.scalar.activation(
                        wq_tiles[t][:], wt[:], Act.Copy, bias=-MAGIC, scale=1.0
                    )

            # ---------- Phase E: y = (x @ Wq^T) * scale ----------
            def evict(ps0, ps1, row):
                yo = yout_pool.tile([P, O_CORE], f32, name="yo")
                nc.vector.tensor_scalar_mul(yo[:, 0:512], ps0[:], scale_t[:])
                nc.vector.tensor_scalar_mul(yo[:, 512:1024], ps1[:], scale_t[:])
                nc.sync.dma_start(y_ap[row : row + P, :], yo[:])

            x_blocks = []
            for nb in range(N_SBLK):
                xb = xbf_pool.tile([P, KP, S_BLK], bf16, tag="xb", name=f"xb{nb}")
                # SWDGE casts fp32 -> bf16 inline during the HBM->SBUF DMA
                xdma = nc.gpsimd.dma_start(
                    xb[:], xT_r[:, :, nb * S_BLK : (nb + 1) * S_BLK]
                )
                # don't let x descriptor-gen delay the gather on the
                # gpsimd queue
                add_dep_helper(xdma.ins, gate.ins, sync=False,
                               reason="x load after gather trigger")
                x_blocks.append(xb)
                if nb >= 2:
                    break  # rest allocated in the steady loop below

            # Fast path: the first 3 s-tiles accumulate in 6 concurrent PSUM
            # banks, consuming wq tiles in (estimated) quantization-completion
            # order so the PE never waits for the full quantize pass.
            fast_units = []  # (psum, s_tile_global, o_half)
            for stg in range(3):
                ps0 = psum_mm.tile([P, 512], f32, tag="mm0", name=f"fps0_{stg}")
                ps1 = psum_mm.tile([P, 512], f32, tag="mm1", name=f"fps1_{stg}")
                fast_units.append((ps0, stg, 0))
                fast_units.append((ps1, stg, 1))
            for ti, t in enumerate(t_order):
                first, last = (ti == 0), (ti == len(t_order) - 1)
                for ps, stg, half in fast_units:
                    xb = x_blocks[stg // 2]
                    s_lo = (stg % 2) * P
                    for kk in range(WCH):
                        k = t * WCH + kk
                        nc.tensor.matmul(
                            ps[:],
                            lhsT=xb[:, k, s_lo : s_lo + P],
                            rhs=wq_tiles[t][:, kk, 512 * half : 512 * (half + 1)],
                            start=first and kk == 0,
                            stop=last and kk == WCH - 1,
                        )
            for stg in range(3):
                evict(fast_units[2 * stg][0], fast_units[2 * stg + 1][0], stg * P)

            # Steady state
            for nb in range(1, N_SBLK):
                if nb >= 3:
                    xb = xbf_pool.tile([P, KP, S_BLK], bf16, tag="xb", name=f"xb{nb}")
                    nc.gpsimd.dma_start(
                        xb[:], xT_r[:, :, nb * S_BLK : (nb + 1) * S_BLK]
                    )
                else:
                    xb = x_blocks[nb]
                for st in range(S_BLK // P):
                    if nb == 1 and st == 0:
                        continue  # covered by the fast path
                    ps0 = psum_mm.tile([P, 512], f32, tag="mm0", name="ps0")
                    ps1 = psum_mm.tile([P, 512], f32, tag="mm1", name="ps1")
                    s_lo = st * P
                    for k in range(KP):
                        lhs = xb[:, k, s_lo : s_lo + P]
                        wqk = wq_tiles[k // WCH][:, k % WCH, :]
                        first, last = (k == 0), (k == KP - 1)
                        nc.tensor.matmul(
                            ps0[:], lhsT=lhs, rhs=wqk[:, 0:512],
                            start=first, stop=last,
                        )
                        nc.tensor.matmul(
                            ps1[:], lhsT=lhs, rhs=wqk[:, 512:1024],
                            start=first, stop=last,
                        )
                    evict(ps0, ps1, nb * S_BLK + s_lo)

    nc.compile()
    return nc


def _get_nc():
    if "nc" not in _nc_cache:
        _nc_cache["nc"] = _build_kernel()
    return _nc_cache["nc"]


def _shard_inputs(x, W):
    x2 = np.ascontiguousarray(np.asarray(x, dtype=np.float32).reshape(S_TOT, I_DIM))
    W2 = np.ascontiguousarray(np.asarray(W, dtype=np.float32))

    xT_slices = [
        np.ascontiguousarray(x2[r * S_CORE : (r + 1) * S_CORE, :].T)
        for r in range(R_CORES)
    ]
    wT_slices = [
        np.ascontiguousarray(W2[c * O_CORE : (c + 1) * O_CORE, :].T)
        for c in range(C_CORES)
    ]
    wred_slices = [
        np.ascontiguousarray(W2[c * W_RED : (c + 1) * W_RED, :].T)
        for c in range(N_CORES)
    ]
    in_maps = []
    for core in range(N_CORES):
        ri, ci = core // C_CORES, core % C_CORES
        in_maps.append(
            {"xT": xT_slices[ri], "wT": wT_slices[ci], "wred": wred_slices[core]}
        )
    return in_maps


def _gather_output(results):
    y = np.empty((S_TOT, O_DIM), dtype=np.float32)
    for core in range(N_CORES):
        ri, ci = core // C_CORES, core % C_CORES
        y[ri * S_CORE : (ri + 1) * S_CORE, ci * O_CORE : (ci + 1) * O_CORE] = (
            results[core]["y"]
        )
    return y.reshape(B, SEQ, O_DIM)


def _run(x, W, **spmd_kwargs):
    import time

    from concourse.bass_utils import run_bass_kernel_spmd

    nc = _get_nc()
    in_maps = _shard_inputs(x, W)
    last_err = None
    for attempt in range(3):
        try:
            res = run_bass_kernel_spmd(
                nc, in_maps, core_ids=list(range(N_CORES)), **spmd_kwargs
            )
            return _gather_output(res.results), res
        except Exception as e:  # transient device wedges recover on retry
            last_err = e
            time.sleep(5.0 * (attempt + 1))
    raise last_err


def kernel(x, W):
    out, _ = _run(x, W)
    return out

